# revision 31
# baseline (speedup 1.0000x reference)
"""Trainium2 Bass kernel for nn_CoreBlock (circulant attention + 2-layer FFN).

Contract: kernel(**inputs) takes FULL unsharded inputs (as produced by
setup_inputs) and returns the FULL [16, 1024, 768] f32 output.

Strategy: pure data-parallel over batch — 8 NeuronCores x 2 batches each,
all weights replicated. v2 design (vs the PE-transpose baseline):

  - Phase A needs x transposed for the value projection; the host supplies
    a pre-transposed bf16 copy of x in DRAM, so the PE does no transposes
    and waits on no LayerNorm stats in phase A.
  - LayerNorm-1 is deferred past the projection: v = (x@Wv' -
    mu*colsum(Wv'))*rstd. The mean comes free from an all-ones column
    appended to Wv' (and a row-sums column appended to Wf for the FFN
    LayerNorms) — column 768 of each matmul output is the row sum.
  - rstd = 1/sqrt(var+eps) runs on the DVE via a bitcast Newton iteration
    (2 steps), so the scalar engine only ever loads the Silu and Exp/Ln
    activation tables (no Sqrt-table thrash).
  - log_cosh tail: |w| + ln(0.5*exp(-2|w|) + 0.5) — the -log2 folds into
    the Ln bias; adds run on the otherwise-idle GPSIMD.
  - Circulant matmuls stream both batches at once (moving free dim 128);
    the Toeplitz bank is fully SBUF-resident.
  - Phase C is staged (layer, batch): scalar/DVE epilogues of one stage
    hide under the next stage's matmuls; the log_cosh tail of batch 0
    hides under layer-1 matmuls of batch 1. Residual x1 is kept in bf16
    so layer-0 transposes run at full PE rate.

Matmul operands are bf16 (full-rate PE, fp32 PSUM accumulation); stats in
fp32.
"""

import math
import numpy as np
import ml_dtypes

import concourse.bass as bass
import concourse.tile as tile
from concourse import bacc, mybir
from concourse.bass_utils import run_bass_kernel_spmd

BF16 = ml_dtypes.bfloat16

B, N, D = 16, 1024, 768
H, HS, L = 12, 64, 2
EPS = 1e-6
NCORES = 8
BPC = B // NCORES          # batches per core
NJ = N // 128              # token chunks per batch (8)
NT = BPC * NJ              # token chunks per core (16)
DC = D // 128              # feature chunks (6)
DW = D + 4                 # matmul width incl. row-sum column (768 + 1, pad 4)
INV_D = 1.0 / D
MAGIC = 0x5F3759DF

F32 = mybir.dt.float32
BF = mybir.dt.bfloat16
I32 = mybir.dt.int32
Alu = mybir.AluOpType
Act = mybir.ActivationFunctionType

TRACE = False              # test harness sets this for profiling runs
TRACE_KW = {}
DEBUG = False

_cache = {}
_MEAN_BF = [0.0] * L


def _emit_rsqrt(nc, magic, tmps, var_ap, rs_ap, n):
    """rs = 1/sqrt(var) on DVE via bitcast Newton (2 steps). var_ap/rs_ap:
    [128, n] f32 SBUF APs. tmps: (bsh_i32, vh_f32, t_f32) tiles >= n wide."""
    bsh, vh, t = tmps
    nc.vector.tensor_scalar(bsh[:, 0:n], var_ap.bitcast(I32), 1, None,
                            op0=Alu.logical_shift_right)
    nc.vector.scalar_tensor_tensor(rs_ap.bitcast(I32), magic[:, 0:n], 0,
                                   bsh[:, 0:n], op0=Alu.bypass,
                                   op1=Alu.subtract)
    nc.vector.tensor_scalar(vh[:, 0:n], var_ap, -0.5, None, op0=Alu.mult)
    for _ in range(2):
        nc.vector.tensor_tensor(t[:, 0:n], rs_ap, rs_ap, op=Alu.mult)
        nc.vector.tensor_tensor(t[:, 0:n], t[:, 0:n], vh[:, 0:n], op=Alu.mult)
        nc.vector.tensor_scalar(t[:, 0:n], t[:, 0:n], 1.5, None, op0=Alu.add)
        nc.vector.tensor_tensor(rs_ap, rs_ap, t[:, 0:n], op=Alu.mult)


def _build(cv_nonzero, bf_nonzero, lnf_uniform, mean_bf):
    global DEBUG
    nc = bacc.Bacc("TRN2", target_bir_lowering=False, debug=False)

    xb_d = nc.dram_tensor("xb", (BPC, N, D), BF, kind="ExternalInput").ap()
    xt_d = nc.dram_tensor("xt", (DC, 128, BPC, N), BF, kind="ExternalInput").ap()
    wv = nc.dram_tensor("wv", (DC, 128, DW), BF, kind="ExternalInput").ap()
    wf = nc.dram_tensor("wf", (L, DC, 128, DW), BF, kind="ExternalInput").ap()
    tb_d = nc.dram_tensor("tbank", (H, 128, NJ * 128), BF, kind="ExternalInput").ap()
    wvsum_d = nc.dram_tensor("wvsum", (128, D), BF, kind="ExternalInput").ap()
    idbf_d = nc.dram_tensor("idbf", (128, 128), BF, kind="ExternalInput").ap()
    cv_d = nc.dram_tensor("cv", (128, D), F32, kind="ExternalInput").ap()
    bf_d = nc.dram_tensor("bfb", (L, 128, D), F32, kind="ExternalInput").ap()
    lnfs_d = nc.dram_tensor("lnfs", (L, 128, D), F32, kind="ExternalInput").ap()
    lnfb_d = nc.dram_tensor("lnfb", (L, 128, D), F32, kind="ExternalInput").ap()
    out_d = nc.dram_tensor("out", (BPC, N, D), F32, kind="ExternalOutput").ap()
    dbg = None
    if DEBUG:
        dbg = {
            "dbg_V": nc.dram_tensor("dbg_V", (128, H, NJ, BPC, HS), BF, kind="ExternalOutput").ap(),
            "dbg_XB": nc.dram_tensor("dbg_XB", (128, BPC, NJ, D), BF, kind="ExternalOutput").ap(),
            "dbg_y0": nc.dram_tensor("dbg_y0", (NT, 128, D), BF, kind="ExternalOutput").ap(),
            "dbg_pv": nc.dram_tensor("dbg_pv", (NT, 128, DW), BF, kind="ExternalOutput").ap(),
            "dbg_rsA": nc.dram_tensor("dbg_rsA", (128, NT), F32, kind="ExternalOutput").ap(),
        }

    with tile.TileContext(nc) as tc:
        _emit(nc, tc, xb_d, xt_d, wv, wf, tb_d, wvsum_d, idbf_d, cv_d, bf_d,
              lnfs_d, lnfb_d, out_d, cv_nonzero, bf_nonzero, lnf_uniform,
              mean_bf, dbg)
    nc.compile()
    return nc


def _emit(nc, tc, xb_d, xt_d, wv, wf, tb_d, wvsum_d, idbf_d, cv_d, bf_d,
          lnfs_d, lnfb_d, out_d, cv_nonzero, bf_nonzero, lnf_uniform,
          mean_bf, dbg=None):
    from contextlib import ExitStack
    ctx = ExitStack()
    with ctx:
        consts = ctx.enter_context(tc.tile_pool(name="consts", bufs=1))
        xpool = ctx.enter_context(tc.tile_pool(name="xpool", bufs=1))
        vpool = ctx.enter_context(tc.tile_pool(name="vpool", bufs=1))
        dtp = ctx.enter_context(tc.tile_pool(name="dtp", bufs=3))
        sbp = ctx.enter_context(tc.tile_pool(name="sbp", bufs=12))   # psum copies
        acts = ctx.enter_context(tc.tile_pool(name="acts", bufs=16))
        scrp = ctx.enter_context(tc.tile_pool(name="scrp", bufs=3))
        stat = ctx.enter_context(tc.tile_pool(name="stat", bufs=1))
        tailp = ctx.enter_context(tc.tile_pool(name="tailp", bufs=3))
        outp = ctx.enter_context(tc.tile_pool(name="outp", bufs=2))
        ps_mm = ctx.enter_context(tc.tile_pool(name="ps_mm", bufs=2, space="PSUM"))
        ps_tr = ctx.enter_context(tc.tile_pool(name="ps_tr", bufs=2, space="PSUM"))
        ps_c = ctx.enter_context(tc.tile_pool(name="ps_c", bufs=2, space="PSUM"))

        # ---- constants / weights (dispatch first; transfers are async) ----
        wv_s = consts.tile([128, DC, DW], BF, tag="wv")
        wvsum_b = consts.tile([128, D], BF, tag="wvsum")
        ibf = consts.tile([128, 128], BF, tag="ibf")
        wf_s = consts.tile([128, L, DC, DW], BF, tag="wf")
        tb_s = consts.tile([128, H, NJ, 128], BF, tag="tb")
        XB = xpool.tile([128, BPC, NJ, D], BF, tag="XB")        # x, then x + y
        V = vpool.tile([128, H, NJ, BPC, HS], BF, tag="V")      # per-head values
        # x loads dispatched interleaved with wv halves: pre-transposed
        # pairs on the sync ring, token-major quarters on the scalar ring
        nc.sync.dma_start(wv_s[:, 0:3, :], wv[0:3].rearrange("c p f -> p c f"))
        udt2 = {}
        for t in range(0, NT, 2):
            b, jc = divmod(t, NJ)
            u2 = dtp.tile([128, DC, 256], BF, tag="udt", bufs=6, name="u2")
            nc.sync.dma_start(
                u2[:], xt_d[:, :, b, jc * 128:(jc + 2) * 128].rearrange(
                    "c p n -> p c n"))
            udt2[t] = u2
            if t == 0:
                nc.sync.dma_start(wv_s[:, 3:DC, :],
                                  wv[3:DC].rearrange("c p f -> p c f"))
        for q in range(4):
            b, j0 = divmod(q * 4, NJ)
            nc.scalar.dma_start(
                XB[:, b, j0:j0 + 4, :],
                xb_d[b, j0 * 128:(j0 + 4) * 128, :].rearrange(
                    "(j p) d -> p j d", p=128))
        nc.sync.dma_start(wvsum_b[:], wvsum_d)
        nc.sync.dma_start(ibf[:], idbf_d)
        nc.scalar.dma_start(wf_s[:], wf.rearrange("l c p f -> p l c f"))
        nc.scalar.dma_start(tb_s[:], tb_d.rearrange("h p (m f) -> p h m f", m=NJ))

        magic = consts.tile([128, 16], I32, tag="magic")
        nc.vector.memset(magic[:], MAGIC)
        zerot = consts.tile([128, 1], F32, tag="zero")
        nc.vector.memset(zerot[:], 0.0)
        halft = consts.tile([128, 1], F32, tag="half")
        nc.vector.memset(halft[:], 0.5)
        cvt = None
        if cv_nonzero:
            cvt = consts.tile([128, D], F32, tag="cv")
            nc.sync.dma_start(cvt[:], cv_d)
        bft = [None] * L
        lnfst = [None] * L
        lnfbt = [None] * L
        for l in range(L):
            if bf_nonzero[l]:
                bft[l] = consts.tile([128, D], F32, tag=f"bf{l}")
                nc.sync.dma_start(bft[l][:], bf_d[l])
            if lnf_uniform[l] is None:
                lnfst[l] = consts.tile([128, D], F32, tag=f"lnfs{l}")
                nc.sync.dma_start(lnfst[l][:], lnfs_d[l])
                lnfbt[l] = consts.tile([128, D], F32, tag=f"lnfb{l}")
                nc.sync.dma_start(lnfbt[l][:], lnfb_d[l])

        # ---- stats tiles ----
        ssqA = stat.tile([128, NT], F32, tag="ssqA")
        nmuA = stat.tile([128, NT], F32, tag="nmuA")            # -mean
        varA = stat.tile([128, NT], F32, tag="varA")
        rsA = stat.tile([128, NT], F32, tag="rsA")
        nt0 = stat.tile([128, 16], I32, tag="nt0")
        nt1 = stat.tile([128, 16], F32, tag="nt1")
        nt2 = stat.tile([128, 16], F32, tag="nt2")
        ntmp = (nt0, nt1, nt2)

        # ================= phase A: project + deferred LN ===============
        agroups = [(0, 4), (4, 4), (8, 4), (12, 2), (14, 2)]
        pvS = {}
        for (t0, AG) in agroups:
            for t in range(t0, t0 + AG):
                b, jc = divmod(t, NJ)
                xbt = XB[:, b, jc, :]
                scr = scrp.tile([128, D], BF, tag="scr")
                nc.scalar.activation(scr[:], xbt, Act.Square,
                                     accum_out=ssqA[:, t:t + 1])
                u2 = udt2[t - t % 2]
                off = (t % 2) * 128
                pv = ps_mm.tile([128, DW], F32, tag="mm")
                for c in range(DC):
                    nc.tensor.matmul(pv[:, 0:512], u2[:, c, off:off + 128],
                                     wv_s[:, c, 0:512],
                                     start=(c == 0), stop=(c == DC - 1))
                    nc.tensor.matmul(pv[:, 512:DW], u2[:, c, off:off + 128],
                                     wv_s[:, c, 512:DW],
                                     start=(c == 0), stop=(c == DC - 1))
                ps = sbp.tile([128, DW], BF, tag="pvS", bufs=3)
                nc.scalar.copy(ps[:], pv[:])
                if dbg is not None:
                    nc.sync.dma_start(dbg["dbg_pv"][t], ps[:])
                pvS[t] = ps
            # group epilogue: -mu, var, rstd on DVE; then fixup + V write
            for t in range(t0, t0 + AG):
                nc.vector.tensor_scalar(nmuA[:, t:t + 1],
                                        pvS[t][:, D:D + 1], -INV_D, None,
                                        op0=Alu.mult)
            g = slice(t0, t0 + AG)
            nc.vector.tensor_scalar(varA[:, g], ssqA[:, g], INV_D, EPS,
                                    op0=Alu.mult, op1=Alu.add)
            nc.vector.scalar_tensor_tensor(rsA[:, g], nmuA[:, g], -1.0,
                                           nmuA[:, g], op0=Alu.mult,
                                           op1=Alu.mult)      # -(mu^2)
            nc.vector.tensor_tensor(varA[:, g], varA[:, g], rsA[:, g],
                                    op=Alu.add)
            _emit_rsqrt(nc, magic, ntmp, varA[:, g], rsA[:, g], AG)
            for t in range(t0, t0 + AG):
                b, jc = divmod(t, NJ)
                tt = scrp.tile([128, D], BF, tag="scr")
                nc.vector.scalar_tensor_tensor(tt[:], wvsum_b[:],
                                               nmuA[:, t:t + 1],
                                               pvS[t][:, 0:D],
                                               op0=Alu.mult, op1=Alu.add)
                vdst = V[:, :, jc, b, :]
                tt3 = tt[:].rearrange("p (h k) -> p h k", h=H)
                if cv_nonzero:
                    cv3 = cvt[:].rearrange("p (h k) -> p h k", h=H)
                    nc.vector.scalar_tensor_tensor(vdst, tt3, rsA[:, t:t + 1],
                                                   cv3, op0=Alu.mult,
                                                   op1=Alu.add)
                else:
                    nc.vector.tensor_scalar(vdst, tt3, rsA[:, t:t + 1], None,
                                            op0=Alu.mult)
                pvS[t] = None

        if dbg is not None:
            nc.sync.dma_start(dbg["dbg_rsA"], rsA[:])
            nc.sync.dma_start(dbg["dbg_V"], V[:])
        # ================= phase B: circulant + residual ================
        for half in range(2):
            i0 = half * 4
            for h in range(H):
                pc = ps_c.tile([128, 4, BPC, HS], F32, tag="pc")
                for m in range(NJ):
                    for i in range(4):
                        jc = (i0 + i + m) % NJ
                        rhs = V[:, h, jc, :, :].rearrange("p b k -> p (b k)")
                        nc.tensor.matmul(
                            pc[:, i, :, :].rearrange("p b k -> p (b k)"),
                            tb_s[:, h, m, :], rhs,
                            start=(m == 0 and i == 0), stop=(m == NJ - 1),
                            skip_group_check=True)
                for b in range(BPC):
                    xap = XB[:, b, i0:i0 + 4, h * HS:(h + 1) * HS]
                    nc.vector.tensor_tensor(xap, xap, pc[:, :, b, :], op=Alu.add)

        if dbg is not None:
            nc.sync.dma_start(dbg["dbg_XB"], XB[:])
        # ================= phase C: FFN x2, staged (layer, batch) =======
        # Stages (l, bb). The silu (+log_cosh tail for l=1) of one stage is
        # deferred and interleaved into the next stage's chunk loop so the
        # scalar queue never blocks the next stage's PSUM-freeing copies.
        ycur = [None] * NT
        statC = {}
        for l in range(L):
            ssqCt = stat.tile([128, NT], F32, tag=f"ssqC{l}")
            nmuCt = stat.tile([128, NT], F32, tag=f"nmuC{l}")
            varCt = stat.tile([128, NT], F32, tag=f"varC{l}")
            rsCt = stat.tile([128, NT], F32, tag=f"rsC{l}")
            biasCt = stat.tile([128, NT], F32, tag=f"biasC{l}")
            statC[l] = dict(ssq=ssqCt, nmu=nmuCt, var=varCt, rs=rsCt,
                            bias=biasCt)

        def emit_silu(l, bb, jc, yraw_t):
            t = bb * NJ + jc
            st = statC[l]
            fast = lnf_uniform[l] is not None
            y = acts.tile([128, D], BF, tag="acts")
            if fast:
                nc.scalar.activation(y[:], yraw_t[:, 0:D], Act.Silu,
                                     bias=st["bias"][:, t:t + 1],
                                     scale=st["rs"][:, t:t + 1])
            else:
                tmp = acts.tile([128, D], BF, tag="acts")
                nc.vector.tensor_scalar(tmp[:], yraw_t[:, 0:D],
                                        st["nmu"][:, t:t + 1],
                                        st["rs"][:, t:t + 1],
                                        op0=Alu.add, op1=Alu.mult)
                nc.vector.tensor_tensor(tmp[:], tmp[:], lnfst[l][:],
                                        op=Alu.mult)
                nc.vector.tensor_tensor(tmp[:], tmp[:], lnfbt[l][:],
                                        op=Alu.add)
                nc.scalar.activation(y[:], tmp[:], Act.Silu, bias=zerot[:])
            if dbg is not None and l == 0:
                nc.sync.dma_start(dbg["dbg_y0"][t], y[:])
            ycur[t] = y

        otile = [None]
        wtile = {}
        etile = {}

        # log_cosh(w) = ln(0.5*exp(2w) + 0.5) - w  (softplus(2w) - ln2 - w)
        def emit_wadd(bb, jc):
            t = bb * NJ + jc
            w = tailp.tile([128, D], BF, tag="w", name="w", bufs=6)
            nc.vector.tensor_tensor(w[:], XB[:, bb, jc, :], ycur[t][:],
                                    op=Alu.add)
            wtile[t] = w

        def emit_exp(bb, jc):
            t = bb * NJ + jc
            e = tailp.tile([128, D], BF, tag="e", name="e", bufs=6)
            nc.scalar.activation(e[:], wtile[t][:], Act.Exp, bias=zerot[:],
                                 scale=2.0)
            etile[t] = e

        def emit_ln(bb, jc):
            t = bb * NJ + jc
            nc.scalar.activation(etile[t][:], etile[t][:], Act.Ln,
                                 bias=halft[:], scale=0.5)

        def emit_final(bb, jc):
            t = bb * NJ + jc
            if jc % 2 == 0:
                otile[0] = outp.tile([128, 2, D], F32, tag="ot", name="ot")
            dst = otile[0][:, jc % 2, :]
            nc.vector.tensor_tensor(dst, etile[t][:], wtile[t][:],
                                    op=Alu.subtract)
            wtile[t] = etile[t] = None
            if jc % 2 == 1:
                nc.sync.dma_start(
                    out_d[bb, (jc - 1) * 128:(jc + 1) * 128, :].rearrange(
                        "(j p) d -> p j d", p=128),
                    otile[0][:])

        deferred = []

        def emit_stats(l, t, yr):
            st = statC[l]
            if bf_nonzero[l]:
                nc.vector.tensor_tensor(yr[:, 0:D], yr[:, 0:D], bft[l][:],
                                        op=Alu.add)
            scr = scrp.tile([128, D], BF, tag="scr")
            nc.vector.scalar_tensor_tensor(scr[:], yr[:, 0:D], 0.0,
                                           yr[:, 0:D], op0=Alu.add,
                                           op1=Alu.mult,
                                           accum_out=st["ssq"][:, t:t + 1])
            nc.vector.tensor_scalar(st["nmu"][:, t:t + 1], yr[:, D:D + 1],
                                    -INV_D, None, op0=Alu.mult)
            if bf_nonzero[l]:
                nc.vector.tensor_scalar(st["nmu"][:, t:t + 1],
                                        st["nmu"][:, t:t + 1],
                                        -mean_bf[l], None, op0=Alu.add)

        # l=0 in batch-size stages; l=1 in half-batch stages so the final
        # flush (which cannot hide under any matmuls) is only 4 chunks deep
        stages = [(0, 0, 0, NJ), (0, 1, 0, NJ),
                  (1, 0, 0, 4), (1, 0, 4, 4),
                  (1, 1, 0, 4), (1, 1, 4, 4)]
        for (l, bb, j0, jn) in stages:
                st = statC[l]
                pending = None      # (t, yraw tile) awaiting stats emission
                yraw = {}
                npop = -(-len(deferred) // jn)      # drain evenly this stage
                for jc in range(j0, j0 + jn):
                    t = bb * NJ + jc
                    src = XB[:, bb, jc, :] if l == 0 else ycur[t][:]
                    ptr = ps_tr.tile([128, D], BF, tag="tr")
                    for c in range(DC):
                        nc.tensor.transpose(ptr[:, c * 128:(c + 1) * 128],
                                            src[:, c * 128:(c + 1) * 128],
                                            ibf[:])
                    zdt = dtp.tile([128, D], BF, tag="zdt")
                    if l == 0:
                        nc.vector.tensor_copy(zdt[:], ptr[:])
                    else:
                        nc.scalar.copy(zdt[:], ptr[:])
                    pf = ps_mm.tile([128, DW], F32, tag="mm")
                    for c in range(DC):
                        nc.tensor.matmul(pf[:, 0:512],
                                         zdt[:, c * 128:(c + 1) * 128],
                                         wf_s[:, l, c, 0:512],
                                         start=(c == 0), stop=(c == DC - 1))
                        nc.tensor.matmul(pf[:, 512:DW],
                                         zdt[:, c * 128:(c + 1) * 128],
                                         wf_s[:, l, c, 512:DW],
                                         start=(c == 0), stop=(c == DC - 1))
                    yr = sbp.tile([128, DW], BF, tag="yraw", bufs=10)
                    if l == 0:
                        nc.scalar.copy(yr[:], pf[:])
                    else:
                        nc.vector.tensor_copy(yr[:], pf[:])
                    yraw[t] = yr
                    for _ in range(npop):
                        if deferred:
                            deferred.pop(0)()
                    if pending is not None:
                        emit_stats(l, *pending)
                    pending = (t, yr)
                emit_stats(l, *pending)
                # stage epilogue: var, rstd, bias on DVE
                g = slice(bb * NJ + j0, bb * NJ + j0 + jn)
                nc.vector.tensor_scalar(st["var"][:, g], st["ssq"][:, g],
                                        INV_D, EPS, op0=Alu.mult, op1=Alu.add)
                nc.vector.scalar_tensor_tensor(st["bias"][:, g],
                                               st["nmu"][:, g], -1.0,
                                               st["nmu"][:, g], op0=Alu.mult,
                                               op1=Alu.mult)
                nc.vector.tensor_tensor(st["var"][:, g], st["var"][:, g],
                                        st["bias"][:, g], op=Alu.add)
                _emit_rsqrt(nc, magic, ntmp, st["var"][:, g], st["rs"][:, g],
                            jn)
                nc.vector.scalar_tensor_tensor(st["bias"][:, g],
                                               st["nmu"][:, g], 0.0,
                                               st["rs"][:, g], op0=Alu.add,
                                               op1=Alu.mult)   # -mu*rs
                if lnf_uniform[l] is not None:
                    cs, cb = lnf_uniform[l]
                    if cs != 1.0:
                        nc.vector.tensor_scalar(st["rs"][:, g], st["rs"][:, g],
                                                float(cs), None, op0=Alu.mult)
                        nc.vector.tensor_scalar(st["bias"][:, g],
                                                st["bias"][:, g],
                                                float(cs), None, op0=Alu.mult)
                    if cb != 0.0:
                        nc.vector.tensor_scalar(st["bias"][:, g],
                                                st["bias"][:, g],
                                                float(cb), None, op0=Alu.add)
                # defer this stage's epilogue work, batched by function so
                # the scalar engine switches activation tables at most twice
                # per stage (silu table <-> exp/ln table)
                jr = range(j0, j0 + jn)
                for jc in jr:
                    yr = yraw[bb * NJ + jc]
                    deferred.append(lambda l=l, bb=bb, jc=jc, yr=yr:
                                    emit_silu(l, bb, jc, yr))
                if l == L - 1:
                    for jc in jr:
                        deferred.append(lambda bb=bb, jc=jc: emit_wadd(bb, jc))
                    for jc in jr:
                        deferred.append(lambda bb=bb, jc=jc: emit_exp(bb, jc))
                    for jc in jr:
                        deferred.append(lambda bb=bb, jc=jc: emit_ln(bb, jc))
                    for jc in jr:
                        deferred.append(lambda bb=bb, jc=jc: emit_final(bb, jc))
        while deferred:
            deferred.pop(0)()


def _prep(inputs):
    x = np.asarray(inputs["x"], np.float32)
    ln1_s = np.asarray(inputs["ln1_scale"], np.float32)
    ln1_b = np.asarray(inputs["ln1_bias"], np.float32)
    Wv = np.asarray(inputs["Wv"], np.float32)
    alpha = np.asarray(inputs["alpha"], np.float32)
    Wf = np.asarray(inputs["Wf"], np.float32)
    bfv = np.asarray(inputs["bf"], np.float32)
    lnf_s = np.asarray(inputs["lnf_scale"], np.float32)
    lnf_b = np.asarray(inputs["lnf_bias"], np.float32)

    Wv_flat = Wv.transpose(1, 0, 2).reshape(D, H * HS)
    Wvp = (ln1_s[:, None] * Wv_flat).astype(BF16)
    cv = (ln1_b @ Wv_flat).astype(np.float32)
    wvsum = Wvp.astype(np.float32).sum(0).astype(BF16)

    wv772 = np.zeros((D, DW), BF16)
    wv772[:, 0:D] = Wvp
    wv772[:, D] = BF16(1.0)
    wv772 = np.ascontiguousarray(wv772.reshape(DC, 128, DW))

    Wfb = Wf.astype(BF16)
    wf772 = np.zeros((L, D, DW), BF16)
    wf772[:, :, 0:D] = Wfb
    wf772[:, :, D] = Wfb.astype(np.float32).sum(2).astype(BF16)
    wf772 = np.ascontiguousarray(wf772.reshape(L, DC, 128, DW))

    ar = alpha[:, (-np.arange(N)) % N]
    ar2 = np.concatenate([ar, ar], axis=1)
    m_ = np.arange(NJ)[:, None, None]
    p_ = np.arange(128)[None, :, None]
    f_ = np.arange(128)[None, None, :]
    T = ar2[:, N + 128 * m_ + p_ - f_]                  # [H, NJ, 128, 128]
    tbank = np.ascontiguousarray(
        T.transpose(0, 2, 1, 3).reshape(H, 128, NJ * 128)).astype(BF16)

    cv_nonzero = bool(np.any(cv))
    bf_nonzero = tuple(bool(np.any(bfv[l])) for l in range(L))
    mean_bf = tuple(float(bfv[l].mean()) for l in range(L))
    lnf_uniform = []
    for l in range(L):
        s, bb = lnf_s[l], lnf_b[l]
        if np.all(s == s[0]) and np.all(bb == bb[0]):
            lnf_uniform.append((float(s[0]), float(bb[0])))
        else:
            lnf_uniform.append(None)
    key = (cv_nonzero, bf_nonzero, tuple(lnf_uniform), mean_bf, DEBUG)

    common = {
        "wv": wv772,
        "wf": wf772,
        "tbank": tbank,
        "wvsum": np.ascontiguousarray(np.broadcast_to(wvsum, (128, D))),
        "idbf": np.eye(128, dtype=BF16),
        "cv": np.ascontiguousarray(np.broadcast_to(cv, (128, D))),
        "bfb": np.ascontiguousarray(np.broadcast_to(bfv[:, None, :], (L, 128, D))),
        "lnfs": np.ascontiguousarray(np.broadcast_to(lnf_s[:, None, :], (L, 128, D))),
        "lnfb": np.ascontiguousarray(np.broadcast_to(lnf_b[:, None, :], (L, 128, D))),
    }
    return x, key, common, (cv_nonzero, bf_nonzero, lnf_uniform, mean_bf)


def kernel(**inputs):
    x, key, common, flags = _prep(inputs)
    if key not in _cache:
        _cache[key] = _build(*flags)
    nc = _cache[key]
    xb_all = x.astype(BF16)                               # [B, N, D]
    # [DC, 128, B, N]: xt[c, p, b, n] = x[b, n, c*128+p]
    xt_all = np.ascontiguousarray(
        xb_all.transpose(2, 0, 1).reshape(DC, 128, B, N))
    in_maps = []
    for i in range(NCORES):
        m = dict(common)
        m["xb"] = np.ascontiguousarray(xb_all[i * BPC:(i + 1) * BPC])
        m["xt"] = np.ascontiguousarray(xt_all[:, :, i * BPC:(i + 1) * BPC, :])
        in_maps.append(m)
    res = run_bass_kernel_spmd(nc, in_maps, core_ids=list(range(NCORES)),
                               trace=TRACE, **TRACE_KW)
    kernel.last_result = res
    out = np.empty((B, N, D), np.float32)
    for i in range(NCORES):
        out[i * BPC:(i + 1) * BPC] = res.results[i]["out"]
    return out


# revision 32
# speedup vs baseline: 1.0552x; 1.0552x over previous
"""Trainium2 Bass kernel for nn_CoreBlock (circulant attention + 2-layer FFN).

Contract: kernel(**inputs) takes FULL unsharded inputs (as produced by
setup_inputs) and returns the FULL [16, 1024, 768] f32 output.

Strategy: pure data-parallel over batch — 8 NeuronCores x 2 batches each,
all weights replicated. v2 design (vs the PE-transpose baseline):

  - Phase A needs x transposed for the value projection; the host supplies
    a pre-transposed bf16 copy of x in DRAM, so the PE does no transposes
    and waits on no LayerNorm stats in phase A.
  - LayerNorm-1 is deferred past the projection: v = (x@Wv' -
    mu*colsum(Wv'))*rstd. The mean comes free from an all-ones column
    appended to Wv' (and a row-sums column appended to Wf for the FFN
    LayerNorms) — column 768 of each matmul output is the row sum.
  - rstd = 1/sqrt(var+eps) runs on the DVE via a bitcast Newton iteration
    (2 steps), so the scalar engine only ever loads the Silu and Exp/Ln
    activation tables (no Sqrt-table thrash).
  - log_cosh tail: |w| + ln(0.5*exp(-2|w|) + 0.5) — the -log2 folds into
    the Ln bias; adds run on the otherwise-idle GPSIMD.
  - Circulant matmuls stream both batches at once (moving free dim 128);
    the Toeplitz bank is fully SBUF-resident.
  - Phase C is staged (layer, batch): scalar/DVE epilogues of one stage
    hide under the next stage's matmuls; the log_cosh tail of batch 0
    hides under layer-1 matmuls of batch 1. Residual x1 is kept in bf16
    so layer-0 transposes run at full PE rate.

Matmul operands are bf16 (full-rate PE, fp32 PSUM accumulation); stats in
fp32.
"""

import math
import numpy as np
import ml_dtypes

import concourse.bass as bass
import concourse.tile as tile
from concourse import bacc, mybir
from concourse.bass_utils import run_bass_kernel_spmd

BF16 = ml_dtypes.bfloat16

B, N, D = 16, 1024, 768
H, HS, L = 12, 64, 2
EPS = 1e-6
NCORES = 8
BPC = B // NCORES          # batches per core
NJ = N // 128              # token chunks per batch (8)
NT = BPC * NJ              # token chunks per core (16)
DC = D // 128              # feature chunks (6)
DW = D + 4                 # matmul width incl. row-sum column (768 + 1, pad 4)
INV_D = 1.0 / D
MAGIC = 0x5F3759DF

F32 = mybir.dt.float32
BF = mybir.dt.bfloat16
I32 = mybir.dt.int32
Alu = mybir.AluOpType
Act = mybir.ActivationFunctionType

TRACE = False              # test harness sets this for profiling runs
TRACE_KW = {}
DEBUG = False

_cache = {}
_MEAN_BF = [0.0] * L


def _emit_rsqrt(nc, magic, tmps, var_ap, rs_ap, n):
    """rs = 1/sqrt(var) on DVE via bitcast Newton (2 steps). var_ap/rs_ap:
    [128, n] f32 SBUF APs. tmps: (bsh_i32, vh_f32, t_f32) tiles >= n wide."""
    bsh, vh, t = tmps
    nc.vector.tensor_scalar(bsh[:, 0:n], var_ap.bitcast(I32), 1, None,
                            op0=Alu.logical_shift_right)
    nc.vector.scalar_tensor_tensor(rs_ap.bitcast(I32), magic[:, 0:n], 0,
                                   bsh[:, 0:n], op0=Alu.bypass,
                                   op1=Alu.subtract)
    nc.vector.tensor_scalar(vh[:, 0:n], var_ap, -0.5, None, op0=Alu.mult)
    for _ in range(2):
        nc.vector.tensor_tensor(t[:, 0:n], rs_ap, rs_ap, op=Alu.mult)
        nc.vector.tensor_tensor(t[:, 0:n], t[:, 0:n], vh[:, 0:n], op=Alu.mult)
        nc.vector.tensor_scalar(t[:, 0:n], t[:, 0:n], 1.5, None, op0=Alu.add)
        nc.vector.tensor_tensor(rs_ap, rs_ap, t[:, 0:n], op=Alu.mult)


def _build(cv_nonzero, bf_nonzero, lnf_uniform, mean_bf):
    global DEBUG
    nc = bacc.Bacc("TRN2", target_bir_lowering=False, debug=False)

    xb_d = nc.dram_tensor("xb", (BPC, N, D), BF, kind="ExternalInput").ap()
    xt_d = nc.dram_tensor("xt", (DC, 128, BPC, N), BF, kind="ExternalInput").ap()
    wv = nc.dram_tensor("wv", (DC, 128, DW), BF, kind="ExternalInput").ap()
    wf = nc.dram_tensor("wf", (L, DC, 128, DW), BF, kind="ExternalInput").ap()
    tb_d = nc.dram_tensor("tbank", (H, 128, NJ * 128), BF, kind="ExternalInput").ap()
    wvsum_d = nc.dram_tensor("wvsum", (128, D), BF, kind="ExternalInput").ap()
    idbf_d = nc.dram_tensor("idbf", (128, 128), BF, kind="ExternalInput").ap()
    cv_d = nc.dram_tensor("cv", (128, D), F32, kind="ExternalInput").ap()
    bf_d = nc.dram_tensor("bfb", (L, 128, D), F32, kind="ExternalInput").ap()
    lnfs_d = nc.dram_tensor("lnfs", (L, 128, D), F32, kind="ExternalInput").ap()
    lnfb_d = nc.dram_tensor("lnfb", (L, 128, D), F32, kind="ExternalInput").ap()
    out_d = nc.dram_tensor("out", (BPC, N, D), F32, kind="ExternalOutput").ap()
    dbg = None
    if DEBUG:
        dbg = {
            "dbg_V": nc.dram_tensor("dbg_V", (128, H, NJ, BPC, HS), BF, kind="ExternalOutput").ap(),
            "dbg_XB": nc.dram_tensor("dbg_XB", (128, BPC, NJ, D), BF, kind="ExternalOutput").ap(),
            "dbg_y0": nc.dram_tensor("dbg_y0", (NT, 128, D), BF, kind="ExternalOutput").ap(),
            "dbg_pv": nc.dram_tensor("dbg_pv", (NT, 128, DW), BF, kind="ExternalOutput").ap(),
            "dbg_rsA": nc.dram_tensor("dbg_rsA", (128, NT), F32, kind="ExternalOutput").ap(),
        }

    with tile.TileContext(nc) as tc:
        _emit(nc, tc, xb_d, xt_d, wv, wf, tb_d, wvsum_d, idbf_d, cv_d, bf_d,
              lnfs_d, lnfb_d, out_d, cv_nonzero, bf_nonzero, lnf_uniform,
              mean_bf, dbg)
    nc.compile()
    return nc


def _emit(nc, tc, xb_d, xt_d, wv, wf, tb_d, wvsum_d, idbf_d, cv_d, bf_d,
          lnfs_d, lnfb_d, out_d, cv_nonzero, bf_nonzero, lnf_uniform,
          mean_bf, dbg=None):
    from contextlib import ExitStack
    ctx = ExitStack()
    with ctx:
        consts = ctx.enter_context(tc.tile_pool(name="consts", bufs=1))
        xpool = ctx.enter_context(tc.tile_pool(name="xpool", bufs=1))
        vpool = ctx.enter_context(tc.tile_pool(name="vpool", bufs=1))
        dtp = ctx.enter_context(tc.tile_pool(name="dtp", bufs=3))
        sbp = ctx.enter_context(tc.tile_pool(name="sbp", bufs=12))   # psum copies
        acts = ctx.enter_context(tc.tile_pool(name="acts", bufs=16))
        scrp = ctx.enter_context(tc.tile_pool(name="scrp", bufs=3))
        stat = ctx.enter_context(tc.tile_pool(name="stat", bufs=1))
        tailp = ctx.enter_context(tc.tile_pool(name="tailp", bufs=3))
        outp = ctx.enter_context(tc.tile_pool(name="outp", bufs=2))
        ps_mm = ctx.enter_context(tc.tile_pool(name="ps_mm", bufs=2, space="PSUM"))
        ps_tr = ctx.enter_context(tc.tile_pool(name="ps_tr", bufs=2, space="PSUM"))
        ps_c = ctx.enter_context(tc.tile_pool(name="ps_c", bufs=2, space="PSUM"))

        # ---- constants / weights (dispatch first; transfers are async) ----
        wv_s = consts.tile([128, DC, DW], BF, tag="wv")
        wvsum_b = consts.tile([128, D], BF, tag="wvsum")
        ibf = consts.tile([128, 128], BF, tag="ibf")
        wf_s = consts.tile([128, L, DC, DW], BF, tag="wf")
        tb_s = consts.tile([128, H, NJ, 128], BF, tag="tb")
        XB = xpool.tile([128, BPC, NJ, D], BF, tag="XB")        # x, then x + y
        V = vpool.tile([128, H, NJ, BPC, HS], BF, tag="V")      # per-head values
        # x loads dispatched interleaved with wv halves: pre-transposed
        # pairs on the sync ring, token-major quarters on the scalar ring
        nc.sync.dma_start(wv_s[:, 0:3, :], wv[0:3].rearrange("c p f -> p c f"))
        udt2 = {}
        for t in range(0, NT, 2):
            b, jc = divmod(t, NJ)
            u2 = dtp.tile([128, DC, 256], BF, tag="udt", bufs=6, name="u2")
            nc.sync.dma_start(
                u2[:], xt_d[:, :, b, jc * 128:(jc + 2) * 128].rearrange(
                    "c p n -> p c n"))
            udt2[t] = u2
            if t == 0:
                nc.sync.dma_start(wv_s[:, 3:DC, :],
                                  wv[3:DC].rearrange("c p f -> p c f"))
        for q in range(4):
            b, j0 = divmod(q * 4, NJ)
            nc.scalar.dma_start(
                XB[:, b, j0:j0 + 4, :],
                xb_d[b, j0 * 128:(j0 + 4) * 128, :].rearrange(
                    "(j p) d -> p j d", p=128))
        nc.sync.dma_start(wvsum_b[:], wvsum_d)
        nc.sync.dma_start(ibf[:], idbf_d)

        magic = consts.tile([128, 16], I32, tag="magic")
        nc.vector.memset(magic[:], MAGIC)
        zerot = consts.tile([128, 1], F32, tag="zero")
        nc.vector.memset(zerot[:], 0.0)
        halft = consts.tile([128, 1], F32, tag="half")
        nc.vector.memset(halft[:], 0.5)
        cvt = None
        if cv_nonzero:
            cvt = consts.tile([128, D], F32, tag="cv")
            nc.sync.dma_start(cvt[:], cv_d)
        bft = [None] * L
        lnfst = [None] * L
        lnfbt = [None] * L
        for l in range(L):
            if bf_nonzero[l]:
                bft[l] = consts.tile([128, D], F32, tag=f"bf{l}")
                nc.sync.dma_start(bft[l][:], bf_d[l])
            if lnf_uniform[l] is None:
                lnfst[l] = consts.tile([128, D], F32, tag=f"lnfs{l}")
                nc.sync.dma_start(lnfst[l][:], lnfs_d[l])
                lnfbt[l] = consts.tile([128, D], F32, tag=f"lnfb{l}")
                nc.sync.dma_start(lnfbt[l][:], lnfb_d[l])

        # ---- stats tiles ----
        ssqA = stat.tile([128, NT], F32, tag="ssqA")
        nmuA = stat.tile([128, NT], F32, tag="nmuA")            # -mean
        varA = stat.tile([128, NT], F32, tag="varA")
        rsA = stat.tile([128, NT], F32, tag="rsA")
        nt0 = stat.tile([128, 16], I32, tag="nt0")
        nt1 = stat.tile([128, 16], F32, tag="nt1")
        nt2 = stat.tile([128, 16], F32, tag="nt2")
        ntmp = (nt0, nt1, nt2)

        # ================= phase A: project + deferred LN ===============
        agroups = [(0, 4), (4, 4), (8, 4), (12, 2), (14, 2)]
        pvS = {}
        for (gi, (t0, AG)) in enumerate(agroups):
            if gi == 1:
                nc.scalar.dma_start(tb_s[:],
                                    tb_d.rearrange("h p (m f) -> p h m f", m=NJ))
            if gi == 2:
                nc.scalar.dma_start(wf_s[:], wf.rearrange("l c p f -> p l c f"))
            for t in range(t0, t0 + AG):
                b, jc = divmod(t, NJ)
                xbt = XB[:, b, jc, :]
                scr = scrp.tile([128, D], BF, tag="scr")
                nc.scalar.activation(scr[:], xbt, Act.Square,
                                     accum_out=ssqA[:, t:t + 1])
                u2 = udt2[t - t % 2]
                off = (t % 2) * 128
                pv = ps_mm.tile([128, DW], F32, tag="mm")
                for c in range(DC):
                    nc.tensor.matmul(pv[:, 0:512], u2[:, c, off:off + 128],
                                     wv_s[:, c, 0:512],
                                     start=(c == 0), stop=(c == DC - 1))
                    nc.tensor.matmul(pv[:, 512:DW], u2[:, c, off:off + 128],
                                     wv_s[:, c, 512:DW],
                                     start=(c == 0), stop=(c == DC - 1))
                ps = sbp.tile([128, DW], BF, tag="pvS", bufs=3)
                nc.scalar.copy(ps[:], pv[:])
                if dbg is not None:
                    nc.sync.dma_start(dbg["dbg_pv"][t], ps[:])
                pvS[t] = ps
            # group epilogue: -mu, var, rstd on DVE; then fixup + V write
            for t in range(t0, t0 + AG):
                nc.vector.tensor_scalar(nmuA[:, t:t + 1],
                                        pvS[t][:, D:D + 1], -INV_D, None,
                                        op0=Alu.mult)
            g = slice(t0, t0 + AG)
            nc.vector.tensor_scalar(varA[:, g], ssqA[:, g], INV_D, EPS,
                                    op0=Alu.mult, op1=Alu.add)
            nc.vector.scalar_tensor_tensor(rsA[:, g], nmuA[:, g], -1.0,
                                           nmuA[:, g], op0=Alu.mult,
                                           op1=Alu.mult)      # -(mu^2)
            nc.vector.tensor_tensor(varA[:, g], varA[:, g], rsA[:, g],
                                    op=Alu.add)
            _emit_rsqrt(nc, magic, ntmp, varA[:, g], rsA[:, g], AG)
            for t in range(t0, t0 + AG):
                b, jc = divmod(t, NJ)
                tt = scrp.tile([128, D], BF, tag="scr")
                nc.vector.scalar_tensor_tensor(tt[:], wvsum_b[:],
                                               nmuA[:, t:t + 1],
                                               pvS[t][:, 0:D],
                                               op0=Alu.mult, op1=Alu.add)
                vdst = V[:, :, jc, b, :]
                tt3 = tt[:].rearrange("p (h k) -> p h k", h=H)
                if cv_nonzero:
                    cv3 = cvt[:].rearrange("p (h k) -> p h k", h=H)
                    nc.vector.scalar_tensor_tensor(vdst, tt3, rsA[:, t:t + 1],
                                                   cv3, op0=Alu.mult,
                                                   op1=Alu.add)
                else:
                    nc.vector.tensor_scalar(vdst, tt3, rsA[:, t:t + 1], None,
                                            op0=Alu.mult)
                pvS[t] = None

        if dbg is not None:
            nc.sync.dma_start(dbg["dbg_rsA"], rsA[:])
            nc.sync.dma_start(dbg["dbg_V"], V[:])
        # ================= phase B: circulant + residual ================
        for half in range(2):
            i0 = half * 4
            for h in range(H):
                pc = ps_c.tile([128, 4, BPC, HS], F32, tag="pc")
                for m in range(NJ):
                    for i in range(4):
                        jc = (i0 + i + m) % NJ
                        rhs = V[:, h, jc, :, :].rearrange("p b k -> p (b k)")
                        nc.tensor.matmul(
                            pc[:, i, :, :].rearrange("p b k -> p (b k)"),
                            tb_s[:, h, m, :], rhs,
                            start=(m == 0 and i == 0), stop=(m == NJ - 1),
                            skip_group_check=True)
                for b in range(BPC):
                    xap = XB[:, b, i0:i0 + 4, h * HS:(h + 1) * HS]
                    nc.vector.tensor_tensor(xap, xap, pc[:, :, b, :], op=Alu.add)

        if dbg is not None:
            nc.sync.dma_start(dbg["dbg_XB"], XB[:])
        # ================= phase C: FFN x2, staged (layer, batch) =======
        # Stages (l, bb). The silu (+log_cosh tail for l=1) of one stage is
        # deferred and interleaved into the next stage's chunk loop so the
        # scalar queue never blocks the next stage's PSUM-freeing copies.
        ycur = [None] * NT
        statC = {}
        for l in range(L):
            ssqCt = stat.tile([128, NT], F32, tag=f"ssqC{l}")
            nmuCt = stat.tile([128, NT], F32, tag=f"nmuC{l}")
            varCt = stat.tile([128, NT], F32, tag=f"varC{l}")
            rsCt = stat.tile([128, NT], F32, tag=f"rsC{l}")
            biasCt = stat.tile([128, NT], F32, tag=f"biasC{l}")
            statC[l] = dict(ssq=ssqCt, nmu=nmuCt, var=varCt, rs=rsCt,
                            bias=biasCt)

        def emit_silu(l, bb, jc, yraw_t):
            t = bb * NJ + jc
            st = statC[l]
            fast = lnf_uniform[l] is not None
            y = acts.tile([128, D], BF, tag="acts")
            if fast:
                nc.scalar.activation(y[:], yraw_t[:, 0:D], Act.Silu,
                                     bias=st["bias"][:, t:t + 1],
                                     scale=st["rs"][:, t:t + 1])
            else:
                tmp = acts.tile([128, D], BF, tag="acts")
                nc.vector.tensor_scalar(tmp[:], yraw_t[:, 0:D],
                                        st["nmu"][:, t:t + 1],
                                        st["rs"][:, t:t + 1],
                                        op0=Alu.add, op1=Alu.mult)
                nc.vector.tensor_tensor(tmp[:], tmp[:], lnfst[l][:],
                                        op=Alu.mult)
                nc.vector.tensor_tensor(tmp[:], tmp[:], lnfbt[l][:],
                                        op=Alu.add)
                nc.scalar.activation(y[:], tmp[:], Act.Silu, bias=zerot[:])
            if dbg is not None and l == 0:
                nc.sync.dma_start(dbg["dbg_y0"][t], y[:])
            ycur[t] = y

        otile = [None]
        wtile = {}
        etile = {}

        # log_cosh(w) = ln(0.5*exp(2w) + 0.5) - w  (softplus(2w) - ln2 - w)
        def emit_wadd(bb, jc, eng=None):
            t = bb * NJ + jc
            w = tailp.tile([128, D], BF, tag="w", name="w", bufs=6)
            eng = eng or nc.gpsimd
            eng.tensor_tensor(w[:], XB[:, bb, jc, :], ycur[t][:], op=Alu.add)
            wtile[t] = w

        def emit_exp(bb, jc):
            t = bb * NJ + jc
            e = tailp.tile([128, D], BF, tag="e", name="e", bufs=6)
            nc.scalar.activation(e[:], wtile[t][:], Act.Exp, bias=zerot[:],
                                 scale=2.0)
            etile[t] = e

        def emit_ln(bb, jc):
            t = bb * NJ + jc
            nc.scalar.activation(etile[t][:], etile[t][:], Act.Ln,
                                 bias=halft[:], scale=0.5)

        def emit_final(bb, jc, eng=None):
            t = bb * NJ + jc
            if jc % 2 == 0:
                otile[0] = outp.tile([128, 2, D], F32, tag="ot", name="ot")
            dst = otile[0][:, jc % 2, :]
            eng = eng or nc.gpsimd
            eng.tensor_tensor(dst, etile[t][:], wtile[t][:], op=Alu.subtract)
            wtile[t] = etile[t] = None
            if jc % 2 == 1:
                nc.sync.dma_start(
                    out_d[bb, (jc - 1) * 128:(jc + 1) * 128, :].rearrange(
                        "(j p) d -> p j d", p=128),
                    otile[0][:])

        deferred = []

        def emit_stats(l, t, yr):
            st = statC[l]
            if bf_nonzero[l]:
                nc.vector.tensor_tensor(yr[:, 0:D], yr[:, 0:D], bft[l][:],
                                        op=Alu.add)
            scr = scrp.tile([128, D], BF, tag="scr")
            nc.vector.scalar_tensor_tensor(scr[:], yr[:, 0:D], 0.0,
                                           yr[:, 0:D], op0=Alu.add,
                                           op1=Alu.mult,
                                           accum_out=st["ssq"][:, t:t + 1])
            nc.vector.tensor_scalar(st["nmu"][:, t:t + 1], yr[:, D:D + 1],
                                    -INV_D, None, op0=Alu.mult)
            if bf_nonzero[l]:
                nc.vector.tensor_scalar(st["nmu"][:, t:t + 1],
                                        st["nmu"][:, t:t + 1],
                                        -mean_bf[l], None, op0=Alu.add)

        # l=0 in batch-size stages; l=1 in half-batch stages so the final
        # flush (which cannot hide under any matmuls) is only 4 chunks deep
        stages = [(0, 0, 0, NJ), (0, 1, 0, NJ),
                  (1, 0, 0, 4), (1, 0, 4, 4),
                  (1, 1, 0, 4), (1, 1, 4, 4)]
        for (l, bb, j0, jn) in stages:
                st = statC[l]
                pending = None      # (t, yraw tile) awaiting stats emission
                yraw = {}
                npop = -(-len(deferred) // jn)      # drain evenly this stage
                for jc in range(j0, j0 + jn):
                    t = bb * NJ + jc
                    src = XB[:, bb, jc, :] if l == 0 else ycur[t][:]
                    ptr = ps_tr.tile([128, D], BF, tag="tr")
                    for c in range(DC):
                        nc.tensor.transpose(ptr[:, c * 128:(c + 1) * 128],
                                            src[:, c * 128:(c + 1) * 128],
                                            ibf[:])
                    zdt = dtp.tile([128, D], BF, tag="zdt")
                    nc.vector.tensor_copy(zdt[:], ptr[:])
                    pf = ps_mm.tile([128, DW], F32, tag="mm")
                    for c in range(DC):
                        nc.tensor.matmul(pf[:, 0:512],
                                         zdt[:, c * 128:(c + 1) * 128],
                                         wf_s[:, l, c, 0:512],
                                         start=(c == 0), stop=(c == DC - 1))
                        nc.tensor.matmul(pf[:, 512:DW],
                                         zdt[:, c * 128:(c + 1) * 128],
                                         wf_s[:, l, c, 512:DW],
                                         start=(c == 0), stop=(c == DC - 1))
                    yr = sbp.tile([128, DW], BF, tag="yraw", bufs=10)
                    if l == 0:
                        nc.scalar.copy(yr[:], pf[:])
                    else:
                        nc.vector.tensor_copy(yr[:], pf[:])
                    yraw[t] = yr
                    for _ in range(npop):
                        if deferred:
                            deferred.pop(0)()
                    if pending is not None:
                        emit_stats(l, *pending)
                    pending = (t, yr)
                emit_stats(l, *pending)
                # stage epilogue: var, rstd, bias on DVE
                g = slice(bb * NJ + j0, bb * NJ + j0 + jn)
                nc.vector.tensor_scalar(st["var"][:, g], st["ssq"][:, g],
                                        INV_D, EPS, op0=Alu.mult, op1=Alu.add)
                nc.vector.scalar_tensor_tensor(st["bias"][:, g],
                                               st["nmu"][:, g], -1.0,
                                               st["nmu"][:, g], op0=Alu.mult,
                                               op1=Alu.mult)
                nc.vector.tensor_tensor(st["var"][:, g], st["var"][:, g],
                                        st["bias"][:, g], op=Alu.add)
                _emit_rsqrt(nc, magic, ntmp, st["var"][:, g], st["rs"][:, g],
                            jn)
                nc.vector.scalar_tensor_tensor(st["bias"][:, g],
                                               st["nmu"][:, g], 0.0,
                                               st["rs"][:, g], op0=Alu.add,
                                               op1=Alu.mult)   # -mu*rs
                if lnf_uniform[l] is not None:
                    cs, cb = lnf_uniform[l]
                    if cs != 1.0:
                        nc.vector.tensor_scalar(st["rs"][:, g], st["rs"][:, g],
                                                float(cs), None, op0=Alu.mult)
                        nc.vector.tensor_scalar(st["bias"][:, g],
                                                st["bias"][:, g],
                                                float(cs), None, op0=Alu.mult)
                    if cb != 0.0:
                        nc.vector.tensor_scalar(st["bias"][:, g],
                                                st["bias"][:, g],
                                                float(cb), None, op0=Alu.add)
                # defer this stage's epilogue work, batched by function so
                # the scalar engine switches activation tables at most twice
                # per stage (silu table <-> exp/ln table)
                jr = range(j0, j0 + jn)
                for jc in jr:
                    yr = yraw[bb * NJ + jc]
                    deferred.append(lambda l=l, bb=bb, jc=jc, yr=yr:
                                    emit_silu(l, bb, jc, yr))
                if l == L - 1:
                    last = (bb == BPC - 1 and j0 + jn == NJ)
                    eng = nc.vector if last else None
                    for jc in jr:
                        deferred.append(lambda bb=bb, jc=jc, e=eng:
                                        emit_wadd(bb, jc, e))
                    for jc in jr:
                        deferred.append(lambda bb=bb, jc=jc: emit_exp(bb, jc))
                    for jc in jr:
                        deferred.append(lambda bb=bb, jc=jc: emit_ln(bb, jc))
                    for jc in jr:
                        deferred.append(lambda bb=bb, jc=jc, e=eng:
                                        emit_final(bb, jc, e))
        while deferred:
            deferred.pop(0)()


def _prep(inputs):
    x = np.asarray(inputs["x"], np.float32)
    ln1_s = np.asarray(inputs["ln1_scale"], np.float32)
    ln1_b = np.asarray(inputs["ln1_bias"], np.float32)
    Wv = np.asarray(inputs["Wv"], np.float32)
    alpha = np.asarray(inputs["alpha"], np.float32)
    Wf = np.asarray(inputs["Wf"], np.float32)
    bfv = np.asarray(inputs["bf"], np.float32)
    lnf_s = np.asarray(inputs["lnf_scale"], np.float32)
    lnf_b = np.asarray(inputs["lnf_bias"], np.float32)

    Wv_flat = Wv.transpose(1, 0, 2).reshape(D, H * HS)
    Wvp = (ln1_s[:, None] * Wv_flat).astype(BF16)
    cv = (ln1_b @ Wv_flat).astype(np.float32)
    wvsum = Wvp.astype(np.float32).sum(0).astype(BF16)

    wv772 = np.zeros((D, DW), BF16)
    wv772[:, 0:D] = Wvp
    wv772[:, D] = BF16(1.0)
    wv772 = np.ascontiguousarray(wv772.reshape(DC, 128, DW))

    Wfb = Wf.astype(BF16)
    wf772 = np.zeros((L, D, DW), BF16)
    wf772[:, :, 0:D] = Wfb
    wf772[:, :, D] = Wfb.astype(np.float32).sum(2).astype(BF16)
    wf772 = np.ascontiguousarray(wf772.reshape(L, DC, 128, DW))

    ar = alpha[:, (-np.arange(N)) % N]
    ar2 = np.concatenate([ar, ar], axis=1)
    m_ = np.arange(NJ)[:, None, None]
    p_ = np.arange(128)[None, :, None]
    f_ = np.arange(128)[None, None, :]
    T = ar2[:, N + 128 * m_ + p_ - f_]                  # [H, NJ, 128, 128]
    tbank = np.ascontiguousarray(
        T.transpose(0, 2, 1, 3).reshape(H, 128, NJ * 128)).astype(BF16)

    cv_nonzero = bool(np.any(cv))
    bf_nonzero = tuple(bool(np.any(bfv[l])) for l in range(L))
    mean_bf = tuple(float(bfv[l].mean()) for l in range(L))
    lnf_uniform = []
    for l in range(L):
        s, bb = lnf_s[l], lnf_b[l]
        if np.all(s == s[0]) and np.all(bb == bb[0]):
            lnf_uniform.append((float(s[0]), float(bb[0])))
        else:
            lnf_uniform.append(None)
    key = (cv_nonzero, bf_nonzero, tuple(lnf_uniform), mean_bf, DEBUG)

    common = {
        "wv": wv772,
        "wf": wf772,
        "tbank": tbank,
        "wvsum": np.ascontiguousarray(np.broadcast_to(wvsum, (128, D))),
        "idbf": np.eye(128, dtype=BF16),
        "cv": np.ascontiguousarray(np.broadcast_to(cv, (128, D))),
        "bfb": np.ascontiguousarray(np.broadcast_to(bfv[:, None, :], (L, 128, D))),
        "lnfs": np.ascontiguousarray(np.broadcast_to(lnf_s[:, None, :], (L, 128, D))),
        "lnfb": np.ascontiguousarray(np.broadcast_to(lnf_b[:, None, :], (L, 128, D))),
    }
    return x, key, common, (cv_nonzero, bf_nonzero, lnf_uniform, mean_bf)


def kernel(**inputs):
    x, key, common, flags = _prep(inputs)
    if key not in _cache:
        _cache[key] = _build(*flags)
    nc = _cache[key]
    xb_all = x.astype(BF16)                               # [B, N, D]
    # [DC, 128, B, N]: xt[c, p, b, n] = x[b, n, c*128+p]
    xt_all = np.ascontiguousarray(
        xb_all.transpose(2, 0, 1).reshape(DC, 128, B, N))
    in_maps = []
    for i in range(NCORES):
        m = dict(common)
        m["xb"] = np.ascontiguousarray(xb_all[i * BPC:(i + 1) * BPC])
        m["xt"] = np.ascontiguousarray(xt_all[:, :, i * BPC:(i + 1) * BPC, :])
        in_maps.append(m)
    res = run_bass_kernel_spmd(nc, in_maps, core_ids=list(range(NCORES)),
                               trace=TRACE, **TRACE_KW)
    kernel.last_result = res
    out = np.empty((B, N, D), np.float32)
    for i in range(NCORES):
        out[i * BPC:(i + 1) * BPC] = res.results[i]["out"]
    return out


# revision 34
# speedup vs baseline: 1.0677x; 1.0119x over previous
"""Trainium2 Bass kernel for nn_CoreBlock (circulant attention + 2-layer FFN).

Contract: kernel(**inputs) takes FULL unsharded inputs (as produced by
setup_inputs) and returns the FULL [16, 1024, 768] f32 output.

Strategy: pure data-parallel over batch — 8 NeuronCores x 2 batches each,
all weights replicated. v2 design (vs the PE-transpose baseline):

  - Phase A needs x transposed for the value projection; the host supplies
    a pre-transposed bf16 copy of x in DRAM, so the PE does no transposes
    and waits on no LayerNorm stats in phase A.
  - LayerNorm-1 is deferred past the projection: v = (x@Wv' -
    mu*colsum(Wv'))*rstd. The mean comes free from an all-ones column
    appended to Wv' (and a row-sums column appended to Wf for the FFN
    LayerNorms) — column 768 of each matmul output is the row sum.
  - rstd = 1/sqrt(var+eps) runs on the DVE via a bitcast Newton iteration
    (2 steps), so the scalar engine only ever loads the Silu and Exp/Ln
    activation tables (no Sqrt-table thrash).
  - log_cosh tail: |w| + ln(0.5*exp(-2|w|) + 0.5) — the -log2 folds into
    the Ln bias; adds run on the otherwise-idle GPSIMD.
  - Circulant matmuls stream both batches at once (moving free dim 128);
    the Toeplitz bank is fully SBUF-resident.
  - Phase C is staged (layer, batch): scalar/DVE epilogues of one stage
    hide under the next stage's matmuls; the log_cosh tail of batch 0
    hides under layer-1 matmuls of batch 1. Residual x1 is kept in bf16
    so layer-0 transposes run at full PE rate.

Matmul operands are bf16 (full-rate PE, fp32 PSUM accumulation); stats in
fp32.
"""

import math
import numpy as np
import ml_dtypes

import concourse.bass as bass
import concourse.tile as tile
from concourse import bacc, mybir
from concourse.bass_utils import run_bass_kernel_spmd

BF16 = ml_dtypes.bfloat16

B, N, D = 16, 1024, 768
H, HS, L = 12, 64, 2
EPS = 1e-6
NCORES = 8
BPC = B // NCORES          # batches per core
NJ = N // 128              # token chunks per batch (8)
NT = BPC * NJ              # token chunks per core (16)
DC = D // 128              # feature chunks (6)
DW = D + 4                 # matmul width incl. row-sum column (768 + 1, pad 4)
INV_D = 1.0 / D
MAGIC = 0x5F3759DF

F32 = mybir.dt.float32
BF = mybir.dt.bfloat16
I32 = mybir.dt.int32
Alu = mybir.AluOpType
Act = mybir.ActivationFunctionType

TRACE = False              # test harness sets this for profiling runs
TRACE_KW = {}
DEBUG = False

_cache = {}
_MEAN_BF = [0.0] * L


def _emit_rsqrt(nc, magic, tmps, var_ap, rs_ap, n):
    """rs = 1/sqrt(var) on DVE via bitcast Newton (2 steps). var_ap/rs_ap:
    [128, n] f32 SBUF APs. tmps: (bsh_i32, vh_f32, t_f32) tiles >= n wide."""
    bsh, vh, t = tmps
    nc.vector.tensor_scalar(bsh[:, 0:n], var_ap.bitcast(I32), 1, None,
                            op0=Alu.logical_shift_right)
    nc.vector.scalar_tensor_tensor(rs_ap.bitcast(I32), magic[:, 0:n], 0,
                                   bsh[:, 0:n], op0=Alu.bypass,
                                   op1=Alu.subtract)
    nc.vector.tensor_scalar(vh[:, 0:n], var_ap, -0.5, None, op0=Alu.mult)
    for _ in range(2):
        nc.vector.tensor_tensor(t[:, 0:n], rs_ap, rs_ap, op=Alu.mult)
        nc.vector.tensor_tensor(t[:, 0:n], t[:, 0:n], vh[:, 0:n], op=Alu.mult)
        nc.vector.tensor_scalar(t[:, 0:n], t[:, 0:n], 1.5, None, op0=Alu.add)
        nc.vector.tensor_tensor(rs_ap, rs_ap, t[:, 0:n], op=Alu.mult)


def _build(cv_nonzero, bf_nonzero, lnf_uniform, mean_bf):
    global DEBUG
    nc = bacc.Bacc("TRN2", target_bir_lowering=False, debug=False)

    xb_d = nc.dram_tensor("xb", (BPC, N, D), BF, kind="ExternalInput").ap()
    xt_d = nc.dram_tensor("xt", (DC, 128, BPC, N), BF, kind="ExternalInput").ap()
    wv = nc.dram_tensor("wv", (DC, 128, DW), BF, kind="ExternalInput").ap()
    wf = nc.dram_tensor("wf", (L, DC, 128, DW), BF, kind="ExternalInput").ap()
    tb_d = nc.dram_tensor("tbank", (H, 128, NJ * 128), BF, kind="ExternalInput").ap()
    wvsum_d = nc.dram_tensor("wvsum", (128, D), BF, kind="ExternalInput").ap()
    idbf_d = nc.dram_tensor("idbf", (128, 128), BF, kind="ExternalInput").ap()
    cv_d = nc.dram_tensor("cv", (128, D), F32, kind="ExternalInput").ap()
    bf_d = nc.dram_tensor("bfb", (L, 128, D), F32, kind="ExternalInput").ap()
    lnfs_d = nc.dram_tensor("lnfs", (L, 128, D), F32, kind="ExternalInput").ap()
    lnfb_d = nc.dram_tensor("lnfb", (L, 128, D), F32, kind="ExternalInput").ap()
    out_d = nc.dram_tensor("out", (BPC, N, D), F32, kind="ExternalOutput").ap()
    dbg = None
    if DEBUG:
        dbg = {
            "dbg_V": nc.dram_tensor("dbg_V", (128, H, NJ, BPC, HS), BF, kind="ExternalOutput").ap(),
            "dbg_XB": nc.dram_tensor("dbg_XB", (128, BPC, NJ, D), BF, kind="ExternalOutput").ap(),
            "dbg_y0": nc.dram_tensor("dbg_y0", (NT, 128, D), BF, kind="ExternalOutput").ap(),
            "dbg_pv": nc.dram_tensor("dbg_pv", (NT, 128, DW), BF, kind="ExternalOutput").ap(),
            "dbg_rsA": nc.dram_tensor("dbg_rsA", (128, NT), F32, kind="ExternalOutput").ap(),
        }

    with tile.TileContext(nc) as tc:
        _emit(nc, tc, xb_d, xt_d, wv, wf, tb_d, wvsum_d, idbf_d, cv_d, bf_d,
              lnfs_d, lnfb_d, out_d, cv_nonzero, bf_nonzero, lnf_uniform,
              mean_bf, dbg)
    nc.compile()
    return nc


def _emit(nc, tc, xb_d, xt_d, wv, wf, tb_d, wvsum_d, idbf_d, cv_d, bf_d,
          lnfs_d, lnfb_d, out_d, cv_nonzero, bf_nonzero, lnf_uniform,
          mean_bf, dbg=None):
    from contextlib import ExitStack
    ctx = ExitStack()
    with ctx:
        consts = ctx.enter_context(tc.tile_pool(name="consts", bufs=1))
        xpool = ctx.enter_context(tc.tile_pool(name="xpool", bufs=1))
        vpool = ctx.enter_context(tc.tile_pool(name="vpool", bufs=1))
        dtp = ctx.enter_context(tc.tile_pool(name="dtp", bufs=3))
        sbp = ctx.enter_context(tc.tile_pool(name="sbp", bufs=12))   # psum copies
        acts = ctx.enter_context(tc.tile_pool(name="acts", bufs=16))
        scrp = ctx.enter_context(tc.tile_pool(name="scrp", bufs=3))
        stat = ctx.enter_context(tc.tile_pool(name="stat", bufs=1))
        tailp = ctx.enter_context(tc.tile_pool(name="tailp", bufs=3))
        outp = ctx.enter_context(tc.tile_pool(name="outp", bufs=2))
        ps_mm = ctx.enter_context(tc.tile_pool(name="ps_mm", bufs=2, space="PSUM"))
        ps_tr = ctx.enter_context(tc.tile_pool(name="ps_tr", bufs=2, space="PSUM"))
        ps_c = ctx.enter_context(tc.tile_pool(name="ps_c", bufs=2, space="PSUM"))

        # ---- constants / weights (dispatch first; transfers are async) ----
        wv_s = consts.tile([128, DC, DW], BF, tag="wv")
        wvsum_b = consts.tile([128, D], BF, tag="wvsum")
        ibf = consts.tile([128, 128], BF, tag="ibf")
        wf_s = consts.tile([128, L, DC, DW], BF, tag="wf")
        tb_s = consts.tile([128, H, NJ, 128], BF, tag="tb")
        XB = xpool.tile([128, BPC, NJ, D], BF, tag="XB")        # x, then x + y
        V = vpool.tile([128, H, NJ, BPC, HS], BF, tag="V")      # per-head values
        # x loads and wv split across both HWDGE rings; tb/wf go via the
        # (otherwise idle) SWDGE path so they don't steal ring bandwidth
        nc.scalar.dma_start(wv_s[:, 0:3, :], wv[0:3].rearrange("c p f -> p c f"))
        nc.sync.dma_start(wv_s[:, 3:DC, :], wv[3:DC].rearrange("c p f -> p c f"))
        udt2 = {}
        for t in range(0, NT, 2):
            b, jc = divmod(t, NJ)
            u2 = dtp.tile([128, DC, 256], BF, tag="udt", bufs=6, name="u2")
            nc.sync.dma_start(
                u2[:], xt_d[:, :, b, jc * 128:(jc + 2) * 128].rearrange(
                    "c p n -> p c n"))
            udt2[t] = u2
        for q in range(4):
            b, j0 = divmod(q * 4, NJ)
            nc.scalar.dma_start(
                XB[:, b, j0:j0 + 4, :],
                xb_d[b, j0 * 128:(j0 + 4) * 128, :].rearrange(
                    "(j p) d -> p j d", p=128))
        nc.sync.dma_start(wvsum_b[:], wvsum_d)
        nc.sync.dma_start(ibf[:], idbf_d)


        magic = consts.tile([128, 16], I32, tag="magic")
        nc.vector.memset(magic[:], MAGIC)
        zerot = consts.tile([128, 1], F32, tag="zero")
        nc.vector.memset(zerot[:], 0.0)
        halft = consts.tile([128, 1], F32, tag="half")
        nc.vector.memset(halft[:], 0.5)
        cvt = None
        if cv_nonzero:
            cvt = consts.tile([128, D], F32, tag="cv")
            nc.sync.dma_start(cvt[:], cv_d)
        bft = [None] * L
        lnfst = [None] * L
        lnfbt = [None] * L
        for l in range(L):
            if bf_nonzero[l]:
                bft[l] = consts.tile([128, D], F32, tag=f"bf{l}")
                nc.sync.dma_start(bft[l][:], bf_d[l])
            if lnf_uniform[l] is None:
                lnfst[l] = consts.tile([128, D], F32, tag=f"lnfs{l}")
                nc.sync.dma_start(lnfst[l][:], lnfs_d[l])
                lnfbt[l] = consts.tile([128, D], F32, tag=f"lnfb{l}")
                nc.sync.dma_start(lnfbt[l][:], lnfb_d[l])

        # ---- stats tiles ----
        ssqA = stat.tile([128, NT], F32, tag="ssqA")
        nmuA = stat.tile([128, NT], F32, tag="nmuA")            # -mean
        varA = stat.tile([128, NT], F32, tag="varA")
        rsA = stat.tile([128, NT], F32, tag="rsA")
        nt0 = stat.tile([128, 16], I32, tag="nt0")
        nt1 = stat.tile([128, 16], F32, tag="nt1")
        nt2 = stat.tile([128, 16], F32, tag="nt2")
        ntmp = (nt0, nt1, nt2)

        # ================= phase A: project + deferred LN ===============
        agroups = [(0, 4), (4, 4), (8, 4), (12, 2), (14, 2)]
        pvS = {}
        for (gi, (t0, AG)) in enumerate(agroups):
            if gi == 1:
                nc.scalar.dma_start(tb_s[:],
                                    tb_d.rearrange("h p (m f) -> p h m f", m=NJ))
            if gi == 2:
                nc.scalar.dma_start(wf_s[:], wf.rearrange("l c p f -> p l c f"))
            for t in range(t0, t0 + AG):
                b, jc = divmod(t, NJ)
                xbt = XB[:, b, jc, :]
                scr = scrp.tile([128, D], BF, tag="scr")
                nc.scalar.activation(scr[:], xbt, Act.Square,
                                     accum_out=ssqA[:, t:t + 1])
                u2 = udt2[t - t % 2]
                off = (t % 2) * 128
                pv = ps_mm.tile([128, DW], F32, tag="mm")
                for c in range(DC):
                    nc.tensor.matmul(pv[:, 0:512], u2[:, c, off:off + 128],
                                     wv_s[:, c, 0:512],
                                     start=(c == 0), stop=(c == DC - 1))
                    nc.tensor.matmul(pv[:, 512:DW], u2[:, c, off:off + 128],
                                     wv_s[:, c, 512:DW],
                                     start=(c == 0), stop=(c == DC - 1))
                ps = sbp.tile([128, DW], BF, tag="pvS", bufs=3)
                nc.scalar.copy(ps[:], pv[:])
                if dbg is not None:
                    nc.sync.dma_start(dbg["dbg_pv"][t], ps[:])
                pvS[t] = ps
            # group epilogue: -mu, var, rstd on DVE; then fixup + V write
            for t in range(t0, t0 + AG):
                nc.vector.tensor_scalar(nmuA[:, t:t + 1],
                                        pvS[t][:, D:D + 1], -INV_D, None,
                                        op0=Alu.mult)
            g = slice(t0, t0 + AG)
            nc.vector.tensor_scalar(varA[:, g], ssqA[:, g], INV_D, EPS,
                                    op0=Alu.mult, op1=Alu.add)
            nc.vector.scalar_tensor_tensor(rsA[:, g], nmuA[:, g], -1.0,
                                           nmuA[:, g], op0=Alu.mult,
                                           op1=Alu.mult)      # -(mu^2)
            nc.vector.tensor_tensor(varA[:, g], varA[:, g], rsA[:, g],
                                    op=Alu.add)
            _emit_rsqrt(nc, magic, ntmp, varA[:, g], rsA[:, g], AG)
            for t in range(t0, t0 + AG):
                b, jc = divmod(t, NJ)
                tt = scrp.tile([128, D], BF, tag="scr")
                nc.vector.scalar_tensor_tensor(tt[:], wvsum_b[:],
                                               nmuA[:, t:t + 1],
                                               pvS[t][:, 0:D],
                                               op0=Alu.mult, op1=Alu.add)
                vdst = V[:, :, jc, b, :]
                tt3 = tt[:].rearrange("p (h k) -> p h k", h=H)
                if cv_nonzero:
                    cv3 = cvt[:].rearrange("p (h k) -> p h k", h=H)
                    nc.vector.scalar_tensor_tensor(vdst, tt3, rsA[:, t:t + 1],
                                                   cv3, op0=Alu.mult,
                                                   op1=Alu.add)
                else:
                    nc.vector.tensor_scalar(vdst, tt3, rsA[:, t:t + 1], None,
                                            op0=Alu.mult)
                pvS[t] = None

        if dbg is not None:
            nc.sync.dma_start(dbg["dbg_rsA"], rsA[:])
            nc.sync.dma_start(dbg["dbg_V"], V[:])
        # ================= phase B: circulant + residual ================
        for half in range(2):
            i0 = half * 4
            for h in range(H):
                pc = ps_c.tile([128, 4, BPC, HS], F32, tag="pc")
                for m in range(NJ):
                    for i in range(4):
                        jc = (i0 + i + m) % NJ
                        rhs = V[:, h, jc, :, :].rearrange("p b k -> p (b k)")
                        nc.tensor.matmul(
                            pc[:, i, :, :].rearrange("p b k -> p (b k)"),
                            tb_s[:, h, m, :], rhs,
                            start=(m == 0 and i == 0), stop=(m == NJ - 1),
                            skip_group_check=True)
                xap = XB[:, :, i0:i0 + 4, h * HS:(h + 1) * HS]
                pcv = pc[:].rearrange("p i b k -> p b i k")
                nc.vector.tensor_tensor(xap, xap, pcv, op=Alu.add)

        if dbg is not None:
            nc.sync.dma_start(dbg["dbg_XB"], XB[:])
        # ================= phase C: FFN x2, staged (layer, batch) =======
        # Stages (l, bb). The silu (+log_cosh tail for l=1) of one stage is
        # deferred and interleaved into the next stage's chunk loop so the
        # scalar queue never blocks the next stage's PSUM-freeing copies.
        ycur = [None] * NT
        statC = {}
        for l in range(L):
            ssqCt = stat.tile([128, NT], F32, tag=f"ssqC{l}")
            nmuCt = stat.tile([128, NT], F32, tag=f"nmuC{l}")
            varCt = stat.tile([128, NT], F32, tag=f"varC{l}")
            rsCt = stat.tile([128, NT], F32, tag=f"rsC{l}")
            biasCt = stat.tile([128, NT], F32, tag=f"biasC{l}")
            statC[l] = dict(ssq=ssqCt, nmu=nmuCt, var=varCt, rs=rsCt,
                            bias=biasCt)

        def emit_silu(l, bb, jc, yraw_t):
            t = bb * NJ + jc
            st = statC[l]
            fast = lnf_uniform[l] is not None
            y = acts.tile([128, D], BF, tag="acts")
            if fast:
                nc.scalar.activation(y[:], yraw_t[:, 0:D], Act.Silu,
                                     bias=st["bias"][:, t:t + 1],
                                     scale=st["rs"][:, t:t + 1])
            else:
                tmp = acts.tile([128, D], BF, tag="acts")
                nc.vector.tensor_scalar(tmp[:], yraw_t[:, 0:D],
                                        st["nmu"][:, t:t + 1],
                                        st["rs"][:, t:t + 1],
                                        op0=Alu.add, op1=Alu.mult)
                nc.vector.tensor_tensor(tmp[:], tmp[:], lnfst[l][:],
                                        op=Alu.mult)
                nc.vector.tensor_tensor(tmp[:], tmp[:], lnfbt[l][:],
                                        op=Alu.add)
                nc.scalar.activation(y[:], tmp[:], Act.Silu, bias=zerot[:])
            if dbg is not None and l == 0:
                nc.sync.dma_start(dbg["dbg_y0"][t], y[:])
            ycur[t] = y

        otile = [None]
        wtile = {}
        etile = {}

        # log_cosh(w) = ln(0.5*exp(2w) + 0.5) - w  (softplus(2w) - ln2 - w)
        def emit_wadd(bb, jc, eng=None):
            t = bb * NJ + jc
            w = tailp.tile([128, D], BF, tag="w", name="w", bufs=6)
            eng = eng or nc.gpsimd
            eng.tensor_tensor(w[:], XB[:, bb, jc, :], ycur[t][:], op=Alu.add)
            wtile[t] = w

        def emit_exp(bb, jc):
            t = bb * NJ + jc
            e = tailp.tile([128, D], BF, tag="e", name="e", bufs=6)
            nc.scalar.activation(e[:], wtile[t][:], Act.Exp, bias=zerot[:],
                                 scale=2.0)
            etile[t] = e

        def emit_ln(bb, jc):
            t = bb * NJ + jc
            nc.scalar.activation(etile[t][:], etile[t][:], Act.Ln,
                                 bias=halft[:], scale=0.5)

        def emit_final(bb, jc, eng=None):
            t = bb * NJ + jc
            if jc % 2 == 0:
                otile[0] = outp.tile([128, 2, D], F32, tag="ot", name="ot")
            dst = otile[0][:, jc % 2, :]
            eng = eng or nc.gpsimd
            eng.tensor_tensor(dst, etile[t][:], wtile[t][:], op=Alu.subtract)
            wtile[t] = etile[t] = None
            if jc % 2 == 1:
                nc.sync.dma_start(
                    out_d[bb, (jc - 1) * 128:(jc + 1) * 128, :].rearrange(
                        "(j p) d -> p j d", p=128),
                    otile[0][:])

        deferred = []

        def emit_stats(l, t, yr):
            st = statC[l]
            if bf_nonzero[l]:
                nc.vector.tensor_tensor(yr[:, 0:D], yr[:, 0:D], bft[l][:],
                                        op=Alu.add)
            scr = scrp.tile([128, D], BF, tag="scr")
            nc.vector.scalar_tensor_tensor(scr[:], yr[:, 0:D], 0.0,
                                           yr[:, 0:D], op0=Alu.add,
                                           op1=Alu.mult,
                                           accum_out=st["ssq"][:, t:t + 1])
            nc.vector.tensor_scalar(st["nmu"][:, t:t + 1], yr[:, D:D + 1],
                                    -INV_D, None, op0=Alu.mult)
            if bf_nonzero[l]:
                nc.vector.tensor_scalar(st["nmu"][:, t:t + 1],
                                        st["nmu"][:, t:t + 1],
                                        -mean_bf[l], None, op0=Alu.add)

        # l=0 in batch-size stages; l=1 in half-batch stages. l0(b1) sits
        # between the l1(b0) halves and l1(b1) so each stage's epilogue +
        # tail work drains under the next stage's matmuls.
        stages = [(0, 0, 0, NJ), (0, 1, 0, NJ),
                  (1, 0, 0, 4), (1, 0, 4, 4),
                  (1, 1, 0, 4), (1, 1, 4, 4)]
        for (l, bb, j0, jn) in stages:
                st = statC[l]
                pending = None      # (t, yraw tile) awaiting stats emission
                yraw = {}
                npop = -(-len(deferred) // jn)      # drain evenly this stage
                for jc in range(j0, j0 + jn):
                    t = bb * NJ + jc
                    for _ in range(npop):
                        if deferred:
                            deferred.pop(0)()
                    src = XB[:, bb, jc, :] if l == 0 else ycur[t][:]
                    ptr = ps_tr.tile([128, D], BF, tag="tr")
                    for c in range(DC):
                        nc.tensor.transpose(ptr[:, c * 128:(c + 1) * 128],
                                            src[:, c * 128:(c + 1) * 128],
                                            ibf[:])
                    zdt = dtp.tile([128, D], BF, tag="zdt")
                    nc.vector.tensor_copy(zdt[:], ptr[:])
                    pf = ps_mm.tile([128, DW], F32, tag="mm")
                    for c in range(DC):
                        nc.tensor.matmul(pf[:, 0:512],
                                         zdt[:, c * 128:(c + 1) * 128],
                                         wf_s[:, l, c, 0:512],
                                         start=(c == 0), stop=(c == DC - 1))
                        nc.tensor.matmul(pf[:, 512:DW],
                                         zdt[:, c * 128:(c + 1) * 128],
                                         wf_s[:, l, c, 512:DW],
                                         start=(c == 0), stop=(c == DC - 1))
                    yr = sbp.tile([128, DW], BF, tag="yraw", bufs=10)
                    if l == 0:
                        nc.scalar.copy(yr[:], pf[:])
                    else:
                        nc.vector.tensor_copy(yr[:], pf[:])
                    yraw[t] = yr
                    if pending is not None:
                        emit_stats(l, *pending)
                    pending = (t, yr)
                emit_stats(l, *pending)
                # stage epilogue: var, rstd, bias on DVE
                g = slice(bb * NJ + j0, bb * NJ + j0 + jn)
                nc.vector.tensor_scalar(st["var"][:, g], st["ssq"][:, g],
                                        INV_D, EPS, op0=Alu.mult, op1=Alu.add)
                nc.vector.scalar_tensor_tensor(st["bias"][:, g],
                                               st["nmu"][:, g], -1.0,
                                               st["nmu"][:, g], op0=Alu.mult,
                                               op1=Alu.mult)
                nc.vector.tensor_tensor(st["var"][:, g], st["var"][:, g],
                                        st["bias"][:, g], op=Alu.add)
                _emit_rsqrt(nc, magic, ntmp, st["var"][:, g], st["rs"][:, g],
                            jn)
                nc.vector.scalar_tensor_tensor(st["bias"][:, g],
                                               st["nmu"][:, g], 0.0,
                                               st["rs"][:, g], op0=Alu.add,
                                               op1=Alu.mult)   # -mu*rs
                if lnf_uniform[l] is not None:
                    cs, cb = lnf_uniform[l]
                    if cs != 1.0:
                        nc.vector.tensor_scalar(st["rs"][:, g], st["rs"][:, g],
                                                float(cs), None, op0=Alu.mult)
                        nc.vector.tensor_scalar(st["bias"][:, g],
                                                st["bias"][:, g],
                                                float(cs), None, op0=Alu.mult)
                    if cb != 0.0:
                        nc.vector.tensor_scalar(st["bias"][:, g],
                                                st["bias"][:, g],
                                                float(cb), None, op0=Alu.add)
                # defer this stage's epilogue work, batched by function so
                # the scalar engine switches activation tables at most twice
                # per stage (silu table <-> exp/ln table)
                jr = range(j0, j0 + jn)
                for jc in jr:
                    yr = yraw[bb * NJ + jc]
                    deferred.append(lambda l=l, bb=bb, jc=jc, yr=yr:
                                    emit_silu(l, bb, jc, yr))
                if l == L - 1:
                    last = (bb == BPC - 1 and j0 + jn == NJ)
                    eng = nc.vector if last else None
                    for jc in jr:
                        deferred.append(lambda bb=bb, jc=jc:
                                        emit_wadd(bb, jc, nc.vector))
                    for jc in jr:
                        deferred.append(lambda bb=bb, jc=jc: emit_exp(bb, jc))
                    for jc in jr:
                        deferred.append(lambda bb=bb, jc=jc: emit_ln(bb, jc))
                    for jc in jr:
                        deferred.append(lambda bb=bb, jc=jc, e=eng:
                                        emit_final(bb, jc, e))
        while deferred:
            deferred.pop(0)()


def _prep(inputs):
    x = np.asarray(inputs["x"], np.float32)
    ln1_s = np.asarray(inputs["ln1_scale"], np.float32)
    ln1_b = np.asarray(inputs["ln1_bias"], np.float32)
    Wv = np.asarray(inputs["Wv"], np.float32)
    alpha = np.asarray(inputs["alpha"], np.float32)
    Wf = np.asarray(inputs["Wf"], np.float32)
    bfv = np.asarray(inputs["bf"], np.float32)
    lnf_s = np.asarray(inputs["lnf_scale"], np.float32)
    lnf_b = np.asarray(inputs["lnf_bias"], np.float32)

    Wv_flat = Wv.transpose(1, 0, 2).reshape(D, H * HS)
    Wvp = (ln1_s[:, None] * Wv_flat).astype(BF16)
    cv = (ln1_b @ Wv_flat).astype(np.float32)
    wvsum = Wvp.astype(np.float32).sum(0).astype(BF16)

    wv772 = np.zeros((D, DW), BF16)
    wv772[:, 0:D] = Wvp
    wv772[:, D] = BF16(1.0)
    wv772 = np.ascontiguousarray(wv772.reshape(DC, 128, DW))

    Wfb = Wf.astype(BF16)
    wf772 = np.zeros((L, D, DW), BF16)
    wf772[:, :, 0:D] = Wfb
    wf772[:, :, D] = Wfb.astype(np.float32).sum(2).astype(BF16)
    wf772 = np.ascontiguousarray(wf772.reshape(L, DC, 128, DW))

    ar = alpha[:, (-np.arange(N)) % N]
    ar2 = np.concatenate([ar, ar], axis=1)
    m_ = np.arange(NJ)[:, None, None]
    p_ = np.arange(128)[None, :, None]
    f_ = np.arange(128)[None, None, :]
    T = ar2[:, N + 128 * m_ + p_ - f_]                  # [H, NJ, 128, 128]
    tbank = np.ascontiguousarray(
        T.transpose(0, 2, 1, 3).reshape(H, 128, NJ * 128)).astype(BF16)

    cv_nonzero = bool(np.any(cv))
    bf_nonzero = tuple(bool(np.any(bfv[l])) for l in range(L))
    mean_bf = tuple(float(bfv[l].mean()) for l in range(L))
    lnf_uniform = []
    for l in range(L):
        s, bb = lnf_s[l], lnf_b[l]
        if np.all(s == s[0]) and np.all(bb == bb[0]):
            lnf_uniform.append((float(s[0]), float(bb[0])))
        else:
            lnf_uniform.append(None)
    key = (cv_nonzero, bf_nonzero, tuple(lnf_uniform), mean_bf, DEBUG)

    common = {
        "wv": wv772,
        "wf": wf772,
        "tbank": tbank,
        "wvsum": np.ascontiguousarray(np.broadcast_to(wvsum, (128, D))),
        "idbf": np.eye(128, dtype=BF16),
        "cv": np.ascontiguousarray(np.broadcast_to(cv, (128, D))),
        "bfb": np.ascontiguousarray(np.broadcast_to(bfv[:, None, :], (L, 128, D))),
        "lnfs": np.ascontiguousarray(np.broadcast_to(lnf_s[:, None, :], (L, 128, D))),
        "lnfb": np.ascontiguousarray(np.broadcast_to(lnf_b[:, None, :], (L, 128, D))),
    }
    return x, key, common, (cv_nonzero, bf_nonzero, lnf_uniform, mean_bf)


def kernel(**inputs):
    x, key, common, flags = _prep(inputs)
    if key not in _cache:
        _cache[key] = _build(*flags)
    nc = _cache[key]
    xb_all = x.astype(BF16)                               # [B, N, D]
    # [DC, 128, B, N]: xt[c, p, b, n] = x[b, n, c*128+p]
    xt_all = np.ascontiguousarray(
        xb_all.transpose(2, 0, 1).reshape(DC, 128, B, N))
    in_maps = []
    for i in range(NCORES):
        m = dict(common)
        m["xb"] = np.ascontiguousarray(xb_all[i * BPC:(i + 1) * BPC])
        m["xt"] = np.ascontiguousarray(xt_all[:, :, i * BPC:(i + 1) * BPC, :])
        in_maps.append(m)
    res = run_bass_kernel_spmd(nc, in_maps, core_ids=list(range(NCORES)),
                               trace=TRACE, **TRACE_KW)
    kernel.last_result = res
    out = np.empty((B, N, D), np.float32)
    for i in range(NCORES):
        out[i * BPC:(i + 1) * BPC] = res.results[i]["out"]
    return out
